# revision 4
# baseline (speedup 1.0000x reference)
"""Distributed Trainium2 Bass kernel: causal multi-head attention block
(QKV proj -> causal softmax attention -> out proj -> residual -> LayerNorm)
tensor-parallel over 16 heads across 8 NeuronCores, with an AllToAll to
switch from head-sharding to sequence-sharding before the output projection.

v5: bf16 compute, software-pipelined schedule, Act engine reserved for the
softmax exp. Softmax normalization is deferred through the output
projection: phase A ships raw ctx rows + the exp-sum row through the
AllToAll (one PSUM->SBUF bf16 copy per tile), and the per-(head,query)
reciprocal scaling is applied to the gathered activations right before
the Wo matmul via a PE broadcast. LayerNorm uses raw moments + fused
(y*rstd+b) tensor_scalar, with a trivial-affine fast path.

Self-contained: callable as kernel(**inputs) with the full unsharded inputs.
"""
import numpy as np
import ml_dtypes

import concourse.bacc as bacc
import concourse.mybir as mybir
import concourse.tile as tile
from concourse.bass_utils import run_bass_kernel_spmd

SEQ = 2048
D = 1024
H = 16
DK = 64
NCORES = 8
HPC = 2                 # heads per core
ROWS = SEQ // NCORES    # 256 output rows per core
QT = 512                # q-tile width
NQT = SEQ // QT         # 4
KCH = 128               # k-chunk
NKC = SEQ // KCH        # 16
NXC = D // 128          # 8 contraction chunks
EPS = 1e-5
NEG = -1e30

F32 = mybir.dt.float32
BF16 = mybir.dt.bfloat16
NP_BF16 = ml_dtypes.bfloat16
EXPF = mybir.ActivationFunctionType.Exp
COPYF = mybir.ActivationFunctionType.Copy

ALL_CORES = [list(range(NCORES))]


def build(loop_reps=None, include_collective=True, phases=('p', 'a', 'w'),
          ln_affine=True, unroll_loop=2):
    """Build the SPMD graph. loop_reps wraps the compute (not the collective)
    in a dynamic loop for hardware timing; the loop body holds `unroll_loop`
    unrolled iterations with alternating activation buffers."""
    nc = bacc.Bacc("TRN2", target_bir_lowering=False, debug=False,
                   num_devices=NCORES)
    unroll = 1 if loop_reps is None else unroll_loop
    if loop_reps is not None:
        assert loop_reps % unroll == 0, (loop_reps, unroll)

    xt_d = nc.dram_tensor("xt", [D, SEQ], BF16, kind="ExternalInput")
    wq_d = nc.dram_tensor("wq", [D, 128], BF16, kind="ExternalInput")
    wk_d = nc.dram_tensor("wk", [D, 128], BF16, kind="ExternalInput")
    wv_d = nc.dram_tensor("wv", [D, 128], BF16, kind="ExternalInput")
    wo_d = nc.dram_tensor("wo", [D, D], BF16, kind="ExternalInput")
    bq_d = nc.dram_tensor("bq", [128, 1], F32, kind="ExternalInput")
    bk_d = nc.dram_tensor("bk", [128, 1], F32, kind="ExternalInput")
    xr_d = nc.dram_tensor("xr", [ROWS, D], F32, kind="ExternalInput")
    mask_d = nc.dram_tensor("mask", [128, 128], BF16, kind="ExternalInput")
    ident_d = nc.dram_tensor("ident", [128, 128], BF16, kind="ExternalInput")
    bsel2_d = nc.dram_tensor("bsel2", [32, 128], BF16, kind="ExternalInput")
    gamma_d = nc.dram_tensor("gamma", [128, D], F32, kind="ExternalInput")
    beta_d = nc.dram_tensor("beta", [128, D], F32, kind="ExternalInput")
    out_d = nc.dram_tensor("out", [ROWS, D], F32, kind="ExternalOutput")

    with tile.TileContext(nc) as tc:
        with (
            tc.tile_pool(name="sb_w", bufs=1) as sb_w,          # weights/constants
            tc.tile_pool(name="sb_act", bufs=1) as sb_act,      # persistent activations
            tc.tile_pool(name="sb_xt", bufs=4) as sb_xt,        # x^T slices
            tc.tile_pool(name="sb_e", bufs=7) as sb_e,          # exp tiles
            tc.tile_pool(name="sb_n", bufs=4) as sb_n,          # ctx export tiles
            tc.tile_pool(name="sb_y", bufs=1) as sb_y,          # epilogue tiles
            tc.tile_pool(name="ps_mm", bufs=2, space="PSUM") as ps_mm,
            tc.tile_pool(name="ps_st", bufs=2, space="PSUM") as ps_st,
            tc.tile_pool(name="ps_ctx", bufs=2, space="PSUM") as ps_ctx,
            tc.tile_pool(name="dram", bufs=1, space="DRAM") as dram,
        ):
            # ---- persistent weight/constant loads (outside any timing loop)
            wq_sb = sb_w.tile([128, NXC, 128], BF16, tag="wq")
            wk_sb = sb_w.tile([128, NXC, 128], BF16, tag="wk")
            wv_sb = sb_w.tile([128, NXC, 128], BF16, tag="wv")
            wo_sb = sb_w.tile([128, NXC, D], BF16, tag="wo")
            bq_sb = sb_w.tile([128, 1], F32, tag="bq")
            bk_sb = sb_w.tile([128, 1], F32, tag="bk")
            mask_sb = sb_w.tile([128, 128], BF16, tag="mask")
            ident_sb = sb_w.tile([128, 128], BF16, tag="ident")
            xr_sb = sb_w.tile([128, 2, D], F32, tag="xr")
            gb_sb = sb_w.tile([128, D], F32, tag="gb")
            bb_sb = sb_w.tile([128, D], F32, tag="bb")
            eps_sb = sb_w.tile([128, 1], F32, tag="eps")
            # per-unroll-parity persistent activation tiles
            vp = [sb_w.tile([128, NKC, HPC, 65], BF16, tag=f"vp{par}",
                            name=f"vp{par}") for par in range(unroll)]
            qt_sb = [sb_act.tile([128, SEQ], BF16, tag=f"qt{par}",
                                 name=f"qt{par}") for par in range(unroll)]
            kt_sb = [sb_act.tile([128, SEQ], BF16, tag=f"kt{par}",
                                 name=f"kt{par}") for par in range(unroll)]
            # recip-broadcast helper for the deferred normalize: lhsT
            # bsel2 [2,128] with bsel2[h, p] = (p // 64 == h), so
            # matmul(bsel2, recs[2, cols]) replicates each head's recip
            # row across that head's 64 partitions.
            bsel2 = sb_w.tile([32, 128], BF16, tag="bsel2")

            nc.sync.dma_start(out=wq_sb[:], in_=wq_d.ap().rearrange("(c p) m -> p c m", p=128))
            nc.sync.dma_start(out=wk_sb[:], in_=wk_d.ap().rearrange("(c p) m -> p c m", p=128))
            nc.sync.dma_start(out=wv_sb[:], in_=wv_d.ap().rearrange("(c p) m -> p c m", p=128))
            nc.sync.dma_start(out=wo_sb[:], in_=wo_d.ap().rearrange("(c p) m -> p c m", p=128))
            nc.sync.dma_start(out=bq_sb[:], in_=bq_d[:])
            nc.sync.dma_start(out=bk_sb[:], in_=bk_d[:])
            nc.sync.dma_start(out=mask_sb[:], in_=mask_d[:])
            nc.sync.dma_start(out=ident_sb[:], in_=ident_d[:])
            nc.sync.dma_start(out=xr_sb[:], in_=xr_d.ap().rearrange("(s p) d -> p s d", p=128))
            nc.sync.dma_start(out=gb_sb[:], in_=gamma_d[:])
            nc.sync.dma_start(out=bb_sb[:], in_=beta_d[:])
            nc.sync.dma_start(out=bsel2[:], in_=bsel2_d[:])
            nc.vector.memset(eps_sb[:], EPS)
            # per-(head, query) softmax denominators; rows 2-31 stay at the
            # preamble-set 1.0 so the K=32 broadcast matmul contracts zeros
            # against harmless recips
            sums_t = [sb_w.tile([32, NCORES, ROWS], BF16, tag=f"sums{par}",
                                name=f"sums{par}") for par in range(unroll)]
            for par in range(unroll):
                nc.vector.memset(sums_t[par][:], 1.0)
                nc.vector.memset(vp[par][:, :, :, 64:65], 1.0)

            # payload rows: 0-127 raw ctx channels, 128-129 the per-head
            # softmax denominators (exp-sum rows)
            a2a_in = dram.tile([NCORES, 130, ROWS], BF16, tag="a2a_in")
            a2a_out = dram.tile([NCORES, 130, ROWS], BF16, tag="a2a_out")

            xt_view = xt_d.ap().rearrange("(c p) s -> p c s", p=128)

            def body(par):
                do_p = "p" in phases
                do_a = "a" in phases
                qt, kt, vpp = qt_sb[par], kt_sb[par], vp[par]

                # 4-deep x^T prefetch: all slice DMAs issued up front
                xt_tiles = []
                if do_p:
                    for st in range(NQT):
                        xt_t = sb_xt.tile([128, NXC, QT], BF16, tag="xt",
                                          name=f"xt{par}_{st}")
                        nc.sync.dma_start(
                            out=xt_t[:],
                            in_=xt_view[:, :, st * QT:(st + 1) * QT])
                        xt_tiles.append(xt_t)

                def emit_p(st):
                    xt_t = xt_tiles[st]
                    q_ps = ps_mm.tile([128, QT], F32, tag="mm",
                                      name=f"qps{par}_{st}")
                    for c in range(NXC):
                        nc.tensor.matmul(q_ps[:], wq_sb[:, c, :], xt_t[:, c, :],
                                         start=(c == 0), stop=(c == NXC - 1))
                    nc.vector.tensor_scalar_add(
                        qt[:, st * QT:(st + 1) * QT], q_ps[:], bq_sb[:])
                    k_ps = ps_mm.tile([128, QT], F32, tag="mm",
                                      name=f"kps{par}_{st}")
                    for c in range(NXC):
                        nc.tensor.matmul(k_ps[:], wk_sb[:, c, :], xt_t[:, c, :],
                                         start=(c == 0), stop=(c == NXC - 1))
                    nc.vector.tensor_scalar_add(
                        kt[:, st * QT:(st + 1) * QT], k_ps[:], bk_sb[:])
                    v_ps = ps_mm.tile([128, QT], F32, tag="mm",
                                      name=f"vps{par}_{st}")
                    for sv in range(QT // 128):
                        for c in range(NXC):
                            nc.tensor.matmul(
                                v_ps[:, sv * 128:(sv + 1) * 128],
                                xt_t[:, c, sv * 128:(sv + 1) * 128],
                                wv_sb[:, c, :], start=(c == 0),
                                stop=(c == NXC - 1), skip_group_check=True)
                    nc.vector.tensor_copy(
                        vpp[:, 4 * st:4 * st + 4, :, 0:64],
                        v_ps[:].rearrange("p (c h d) -> p c h d", c=4, h=HPC))

                pend_ctx = []   # delayed ctx-matmul closures
                pend_norm = []  # deferred normalize closures

                def flush_ctx(keep=0):
                    while len(pend_ctx) > keep:
                        pend_ctx.pop(0)()

                def flush_norm():
                    while pend_norm:
                        pend_norm.pop(0)()

                def emit_a(qi):
                    nkc_q = 4 * (qi + 1)   # causal: chunks 0..nkc_q-1
                    ctx_ps = [ps_ctx.tile([65, QT], F32, tag="ctx",
                                          name=f"ctx{par}_{qi}_{h}")
                              for h in range(HPC)]
                    # units: pairs of full chunks, then 2 merged diagonal pairs
                    units = [("pair", c0) for c0 in range(0, 4 * qi, 2)]
                    units += [("dpair", 4 * qi), ("dpair", 4 * qi + 2)]
                    for ui, (kind, c0) in enumerate(units):
                        for h in range(HPC):
                            st2 = ps_st.tile([128, 2 * QT], F32, tag="st",
                                             name=f"st{par}_{qi}_{c0}_{h}")
                            e2 = sb_e.tile([128, 2 * QT], BF16, tag="e",
                                           name=f"e{par}_{qi}_{c0}_{h}")
                            if kind == "pair":
                                for half in range(2):
                                    nc.tensor.matmul(
                                        st2[:, half * QT:(half + 1) * QT],
                                        kt[h * 64:(h + 1) * 64,
                                           (c0 + half) * KCH:(c0 + half + 1) * KCH],
                                        qt[h * 64:(h + 1) * 64,
                                           qi * QT:(qi + 1) * QT],
                                        start=True, stop=True)
                                nc.scalar.activation(e2[:], st2[:], EXPF,
                                                     scale=1.0 / 8.0)

                                def ctx_pair(h=h, c0=c0, e2=e2, cps=ctx_ps):
                                    for half in range(2):
                                        nc.tensor.matmul(
                                            cps[h][:],
                                            vpp[:, c0 + half, h, :],
                                            e2[:, half * QT:(half + 1) * QT],
                                            start=(c0 + half == 0), stop=False,
                                            skip_group_check=True)
                                pend_ctx.append(ctx_pair)
                            else:
                                # merged diagonal pair: chunks c0, c0+1 with
                                # trimmed widths, masks accumulated on the PE
                                subs = []
                                off = 0
                                for ci in (c0, c0 + 1):
                                    qs = ci * KCH - qi * QT
                                    cols = QT - qs
                                    subs.append((ci, qs, cols, off))
                                    off += cols
                                tot = off
                                for ci, qs, cols, off_ in subs:
                                    nc.tensor.matmul(
                                        st2[:, off_:off_ + cols],
                                        kt[h * 64:(h + 1) * 64,
                                           ci * KCH:(ci + 1) * KCH],
                                        qt[h * 64:(h + 1) * 64,
                                           qi * QT + qs:(qi + 1) * QT],
                                        start=True, stop=False,
                                        skip_group_check=True)
                                    nc.tensor.matmul(
                                        st2[:, off_:off_ + 128], ident_sb[:],
                                        mask_sb[:],
                                        start=False, stop=(ci == c0 + 1),
                                        skip_group_check=True)
                                nc.scalar.activation(e2[:, 0:tot], st2[:, 0:tot],
                                                     EXPF, scale=1.0 / 8.0)

                                def ctx_dpair(h=h, subs=subs, e2=e2, cps=ctx_ps,
                                              nkc_q=nkc_q):
                                    for ci, qs, cols, off_ in subs:
                                        nc.tensor.matmul(
                                            cps[h][:, qs:QT],
                                            vpp[:, ci, h, :],
                                            e2[:, off_:off_ + cols],
                                            start=(ci == 0),
                                            stop=(ci == nkc_q - 1),
                                            skip_group_check=True)
                                pend_ctx.append(ctx_dpair)
                        flush_ctx(keep=4)
                    flush_ctx(keep=0)

                    # export raw ctx + sums rows into a2a_in (deferred so the
                    # copy never head-of-line blocks the DVE queue)
                    def norm(qi=qi, ctx_ps=ctx_ps):
                        for h in range(HPC):
                            cx = sb_n.tile([65, QT], BF16, tag="cx",
                                           name=f"cx{par}_{qi}_{h}")
                            nc.vector.tensor_copy(cx[:], ctx_ps[h][:])
                            nc.sync.dma_start(
                                out=a2a_in[2 * qi:2 * qi + 2,
                                           h * 64:(h + 1) * 64, :]
                                    .rearrange("b d q -> d b q"),
                                in_=cx[0:64, :].rearrange("d (b q) -> d b q", b=2))
                            nc.sync.dma_start(
                                out=a2a_in[2 * qi:2 * qi + 2,
                                           128 + h:129 + h, :]
                                    .rearrange("b o q -> o b q"),
                                in_=cx[64:65, :].rearrange("o (b q) -> o b q", b=2))
                    pend_norm.append(norm)

                for st in range(NQT):
                    if do_p:
                        emit_p(st)
                    if do_a:
                        flush_norm()
                        emit_a(st)
                if do_a:
                    flush_norm()

            def tail_body(par):
                if "w" not in phases:
                    return
                # ---- phase W: Wo matmul on own 256 rows + residual + LayerNorm
                # (two independent per-qs chains, interleaved emission;
                #  LayerNorm via raw moments: var = E[y^2] - mu^2)
                ao = sb_act.tile([128, NCORES, ROWS], BF16, tag=f"ao{par}",
                                 name=f"ao{par}")
                nc.sync.dma_start(out=ao[:],
                                  in_=a2a_out[:, 0:128, :].rearrange("j p q -> p j q"))
                # per-(head, query) softmax denominators from all 8 src cores
                sums = sums_t[par]
                nc.sync.dma_start(out=sums[0:2],
                                  in_=a2a_out[:, 128:130, :].rearrange("j p q -> p j q"))
                recs = sb_act.tile([32, NCORES, ROWS], BF16, tag=f"recs{par}",
                                   name=f"recs{par}")
                for half in range(2):
                    with nc.allow_low_precision("softmax recip bf16"):
                        nc.vector.reciprocal(recs[:, 4 * half:4 * half + 4, :],
                                             sums[:, 4 * half:4 * half + 4, :])
                # broadcast each head's recip row across its 64 partitions
                # and scale the gathered activations before Wo
                ao_n = sb_act.tile([128, NCORES, ROWS], BF16, tag=f"aon{par}",
                                   name=f"aon{par}")
                recs_flat = recs[:].rearrange("p j q -> p (j q)")
                for half in range(2):
                    m_ps = ps_st.tile([128, 2 * QT], F32, tag="st",
                                      name=f"mps{par}_{half}")
                    for qtr in range(2):
                        nc.tensor.matmul(
                            m_ps[:, qtr * QT:(qtr + 1) * QT],
                            bsel2[:],
                            recs_flat[:, half * 1024 + qtr * QT:
                                      half * 1024 + (qtr + 1) * QT],
                            start=True, stop=True)
                    nc.vector.tensor_tensor(
                        ao_n[:, 4 * half:4 * half + 4, :],
                        ao[:, 4 * half:4 * half + 4, :],
                        m_ps[:].rearrange("p (j q) -> p j q", j=4),
                        op=mybir.AluOpType.mult)
                t = {}
                for qs in range(2):
                    for nm, shape in (("y", [128, D]), ("sq", [128, D]),
                                      ("sc", [128, D]), ("musum", [128, 1]),
                                      ("numu", [128, 1]), ("m2", [128, 1]),
                                      ("bln", [128, 1]), ("ssq", [128, 1]),
                                      ("lnv", [128, 1]), ("rstd", [128, 1]),
                                      ("b2", [128, 1])):
                        t[nm, qs] = sb_y.tile(shape, F32, tag=f"{nm}{qs}_{par}",
                                              name=f"{nm}{qs}_{par}")
                for ot in range(2):
                    for qs in range(2):
                        y_ps = ps_mm.tile([128, QT], F32, tag="mm",
                                          name=f"yps{par}_{qs}_{ot}")
                        for j in range(NCORES):
                            nc.tensor.matmul(y_ps[:], ao_n[:, j, qs * 128:(qs + 1) * 128],
                                             wo_sb[:, j, ot * QT:(ot + 1) * QT],
                                             start=(j == 0), stop=(j == NCORES - 1))
                        nc.vector.tensor_add(t["y", qs][:, ot * QT:(ot + 1) * QT],
                                             y_ps[:],
                                             xr_sb[:, qs, ot * QT:(ot + 1) * QT])
                for qs in range(2):
                    nc.vector.reduce_sum(t["musum", qs][:], t["y", qs][:],
                                         axis=mybir.AxisListType.X)
                for qs in range(2):
                    # E[y^2]*D on the Act engine, concurrent with the mean path
                    nc.scalar.activation(t["sq", qs][:], t["y", qs][:],
                                         mybir.ActivationFunctionType.Square,
                                         accum_out=t["ssq", qs][:])
                for qs in range(2):
                    nc.scalar.mul(t["numu", qs][:], t["musum", qs][:], -1.0 / D)
                for qs in range(2):
                    nc.vector.tensor_tensor(t["m2", qs][:], t["numu", qs][:],
                                            t["numu", qs][:],
                                            op=mybir.AluOpType.mult)
                for qs in range(2):
                    nc.vector.tensor_tensor(t["bln", qs][:], eps_sb[:],
                                            t["m2", qs][:],
                                            op=mybir.AluOpType.subtract)
                for qs in range(2):
                    # ln(var + eps) = ln(ssq/D + (eps - mu^2))
                    nc.scalar.activation(t["lnv", qs][:], t["ssq", qs][:],
                                         mybir.ActivationFunctionType.Ln,
                                         scale=1.0 / D, bias=t["bln", qs][:])
                for qs in range(2):
                    nc.scalar.activation(t["rstd", qs][:], t["lnv", qs][:],
                                         EXPF, scale=-0.5)
                for qs in range(2):
                    nc.vector.tensor_tensor(t["b2", qs][:], t["numu", qs][:],
                                            t["rstd", qs][:],
                                            op=mybir.AluOpType.mult)
                for qs in range(2):
                    # (y - mu) * rstd = y*rstd + (-mu*rstd)
                    nc.vector.tensor_scalar(t["sc", qs][:], t["y", qs][:],
                                            t["rstd", qs][:], t["b2", qs][:],
                                            op0=mybir.AluOpType.mult,
                                            op1=mybir.AluOpType.add)
                if ln_affine:
                    for qs in range(2):
                        nc.vector.tensor_tensor(t["sc", qs][:], t["sc", qs][:],
                                                gb_sb[:],
                                                op=mybir.AluOpType.mult)
                    for qs in range(2):
                        nc.vector.tensor_add(t["sc", qs][:], t["sc", qs][:],
                                             bb_sb[:])
                for qs in range(2):
                    nc.sync.dma_start(out=out_d[qs * 128:(qs + 1) * 128, :],
                                      in_=t["sc", qs][:])

            if loop_reps is None:
                body(0)
                if include_collective:
                    nc.gpsimd.collective_compute(
                        "AllToAll", mybir.AluOpType.bypass,
                        ins=[a2a_in.opt()], outs=[a2a_out.opt()],
                        replica_groups=ALL_CORES)
                tail_body(0)
            else:
                def full(_i):
                    # bodies back-to-back so the second iteration's
                    # projection work feeds the Act exp stream without
                    # queueing behind the first epilogue's DVE chain
                    for par in range(unroll):
                        body(par)
                    for par in range(unroll):
                        tail_body(par)
                    if not (("p" in phases) or ("a" in phases) or ("w" in phases)):
                        nc.vector.memset(eps_sb[:], EPS)
                with tc.For_i(0, loop_reps // unroll, 1) as i:
                    full(i)

    nc.compile()
    return nc


def _bf16(a):
    return np.ascontiguousarray(np.asarray(a, np.float32).astype(NP_BF16))


def make_in_maps(x, Wq, bq, Wk, bk, Wv, bv, Wo, bo, gamma, beta):
    x = np.asarray(x, np.float32)
    xt_bf = _bf16(x.T)
    kk = np.arange(128, dtype=np.int64)[:, None]
    qq = np.arange(128, dtype=np.int64)[None, :]
    mask = _bf16(np.where(kk <= qq, 0.0, NEG))
    ident = _bf16(np.eye(128, dtype=np.float32))
    bsel2_np = _bf16((np.arange(128)[None, :] // 64) == np.arange(32)[:, None])
    Wo_c = np.ascontiguousarray(np.asarray(Wo, np.float32))
    Wo_bf = _bf16(Wo_c)
    # bv passes through softmax-weighted sums unchanged (rows sum to 1),
    # so its contribution to y is the constant row bv @ Wo; fold into bo.
    bo_eff = (np.asarray(bo, np.float32)
              + np.asarray(bv, np.float32) @ Wo_c).astype(np.float32)
    gamma_b = np.ascontiguousarray(
        np.broadcast_to(np.asarray(gamma, np.float32).reshape(1, D), (128, D)))
    beta_b = np.ascontiguousarray(
        np.broadcast_to(np.asarray(beta, np.float32).reshape(1, D), (128, D)))
    in_maps = []
    for i in range(NCORES):
        cs = slice(128 * i, 128 * (i + 1))
        rs = slice(ROWS * i, ROWS * (i + 1))
        in_maps.append({
            "xt": xt_bf,
            "wq": _bf16(np.asarray(Wq, np.float32)[:, cs]),
            "wk": _bf16(np.asarray(Wk, np.float32)[:, cs]),
            "wv": _bf16(np.asarray(Wv, np.float32)[:, cs]),
            "wo": Wo_bf,
            "bq": np.ascontiguousarray(np.asarray(bq, np.float32)[cs]).reshape(128, 1),
            "bk": np.ascontiguousarray(np.asarray(bk, np.float32)[cs]).reshape(128, 1),
            "xr": np.ascontiguousarray(x[rs, :] + bo_eff),
            "mask": mask,
            "ident": ident,
            "bsel2": bsel2_np,
            "gamma": gamma_b,
            "beta": beta_b,
        })
    return in_maps


_nc_cache = {}
_ln_affine_default = True


def get_nc(loop_reps=None, include_collective=True, phases=("p", "a", "w"),
           ln_affine=None, unroll_loop=2):
    if ln_affine is None:
        ln_affine = _ln_affine_default
    key = (loop_reps, include_collective, tuple(phases), ln_affine, unroll_loop)
    if key not in _nc_cache:
        _nc_cache[key] = build(loop_reps, include_collective, phases=phases,
                               ln_affine=ln_affine, unroll_loop=unroll_loop)
    return _nc_cache[key]


def kernel(x, Wq, bq, Wk, bk, Wv, bv, Wo, bo, gamma, beta):
    global _ln_affine_default
    gamma_np = np.asarray(gamma, np.float32)
    beta_np = np.asarray(beta, np.float32)
    _ln_affine_default = not (np.all(gamma_np == 1.0) and np.all(beta_np == 0.0))
    nc = get_nc()
    in_maps = make_in_maps(x, Wq, bq, Wk, bk, Wv, bv, Wo, bo, gamma, beta)
    res = run_bass_kernel_spmd(nc, in_maps, core_ids=list(range(NCORES)))
    out = np.concatenate([res.results[i]["out"] for i in range(NCORES)], axis=0)
    return np.ascontiguousarray(out.astype(np.float32))
